# revision 2
# baseline (speedup 1.0000x reference)
"""Trainium2 Bass kernel for nn_DenseExpert (soft-gated mixture of dense experts).

Math:  out[b,u] = sum_e gate[b,e] * (x[b,:] @ alpha[e]) [u] + (gate @ beta)[b,u]

Strategy (pure data parallel over batch, 8 cores; BLOC=8192 rows/core).
The bias term (gate @ beta, 1.5% of FLOPs) is added on the host.

Per core, per 512-row group (16 groups of 4 tiles x 128 rows):
  1. DVE: gate-scale x into all 8 expert copies, one tensor_tensor per
     128-row tile: xe[q, e, t, i] = x[128T+q, i] * gate[128T+q, e].
     The gate operand is host-duplicated in pairs (j-dim) so every
     operand's innermost AP stride is 1 -> DVE 2x_1p mode (2 elem/lane/cyc).
  2. DMA xbar transpose (off the compute engines entirely):
     xe [q, (e,t)*128+i] -> xeT [i, (e,t), q]  (batched 128x128 block
     transposes, one instruction per group, 14ns per 16x128 xbar tile).
  3. PE: 8 accumulating matmuls per group (the only matmuls in the
     kernel): oT[u, (t,q)] += alpha_e[i,u].T @ xeT[i, (t,q)], N=512.
  4. ACT: PSUM -> SBUF eviction with f32->f16 cast; out DMA per group.

Host: x/gate cast to f16, gate packed per-partition with j-duplication,
alpha pre-transposed to [i, e, u] f16; output [u, b] f16 transposed back,
cast to f32, bias gate@beta added.
"""

import dataclasses
from contextlib import ExitStack

import numpy as np

import concourse.bacc as bacc
import concourse.tile as tile
import concourse.mybir as mybir
from concourse.bass_utils import run_bass_kernel_spmd

F32 = mybir.dt.float32
F16 = mybir.dt.float16

B, E, I, U = 65536, 8, 128, 128
NCORES = 8
BLOC = B // NCORES        # 8192 rows per core
NT = BLOC // 128          # 64 tiles of 128 rows
GT = 4                    # tiles per group
NG = NT // GT             # 16 groups
GROUP_COLS = GT * 128     # 512 columns of out.T per group

# experts handled by the Pool (gpsimd) engine per tile; rest on DVE
POOL_E = 2


def _build():
    nc = bacc.Bacc("TRN2", target_bir_lowering=False, debug=False)

    x16 = nc.dram_tensor("x16", [BLOC, I], F16, kind="ExternalInput").ap()
    # g2[q, T, e, j] = gate[128T+q, e] duplicated over j in {0,1}
    g2 = nc.dram_tensor("g2", [128, NT, E, 2], F16, kind="ExternalInput").ap()
    # al[i, e, u] = alpha[e, i, u]
    al = nc.dram_tensor("al", [128, E, U], F16, kind="ExternalInput").ap()
    # out.T, columns ordered (T, q): col = 128T + q = batch row
    outT = nc.dram_tensor("outT", [U, BLOC], F16, kind="ExternalOutput").ap()

    with tile.TileContext(nc) as tc, ExitStack() as ctx:
        const = ctx.enter_context(tc.tile_pool(name="const", bufs=1))
        xep = ctx.enter_context(tc.tile_pool(name="xep", bufs=3))
        xtp = ctx.enter_context(tc.tile_pool(name="xtp", bufs=3))
        osp = ctx.enter_context(tc.tile_pool(name="osp", bufs=2))
        ps = ctx.enter_context(tc.tile_pool(name="ps", bufs=2, space="PSUM"))

        x_all = const.tile([128, NT, I], F16, tag="x")
        for q4 in range(4):
            nc.sync.dma_start(
                x_all[:, q4 * 16:(q4 + 1) * 16, :],
                x16[q4 * 2048:(q4 + 1) * 2048, :].rearrange(
                    "(T p) i -> p T i", p=128
                ),
            )
        g_sb = const.tile([128, NT, E, 2], F16, tag="g")
        nc.sync.dma_start(g_sb[:], g2)
        al_sb = const.tile([128, E, U], F16, tag="al")
        nc.sync.dma_start(al_sb[:], al)

        g_pitch = NT * E * 2
        for G in range(NG):
            # 1. gate-scale: xe[q, e, t, i]  (memory layout [e, t, i])
            xe = xep.tile([128, E, GT, I], F16, tag="xe")
            for t in range(GT):
                T = G * GT + t
                # enumerate (e, i2, j): every operand ends stride 1 -> 2x mode
                xe_v = dataclasses.replace(
                    xe[:],
                    ap=[[E * GT * I, 128], [GT * I, E - POOL_E], [2, 64], [1, 2]],
                    offset=t * I,
                )
                x_v = dataclasses.replace(
                    x_all[:],
                    ap=[[NT * I, 128], [0, E - POOL_E], [2, 64], [1, 2]],
                    offset=T * I,
                )
                g_v = dataclasses.replace(
                    g_sb[:],
                    ap=[[g_pitch, 128], [2, E - POOL_E], [0, 64], [1, 2]],
                    offset=T * E * 2,
                )
                nc.vector.tensor_tensor(xe_v, x_v, g_v, op=mybir.AluOpType.mult)
                if POOL_E:
                    xe_p = dataclasses.replace(
                        xe[:],
                        ap=[[E * GT * I, 128], [GT * I, POOL_E], [2, 64], [1, 2]],
                        offset=(E - POOL_E) * GT * I + t * I,
                    )
                    x_p = dataclasses.replace(
                        x_all[:],
                        ap=[[NT * I, 128], [0, POOL_E], [2, 64], [1, 2]],
                        offset=T * I,
                    )
                    g_p = dataclasses.replace(
                        g_sb[:],
                        ap=[[g_pitch, 128], [2, POOL_E], [0, 64], [1, 2]],
                        offset=T * E * 2 + (E - POOL_E) * 2,
                    )
                    nc.gpsimd.tensor_tensor(xe_p, x_p, g_p, op=mybir.AluOpType.mult)

            # 2. xbar transpose: xeT[i, (e,t), q]
            xeT = xtp.tile([128, E * GT, 128], F16, tag="xeT")
            xe2d = dataclasses.replace(
                xe[:], ap=[[E * GT * I, 128], [1, E * GT * I]], offset=0
            )
            nc.sync.dma_start_transpose(xeT[:], xe2d)

            # 3. mains: oT[u, (t, q)] = sum_e alpha_e.T @ xeT_e
            oT = ps.tile([128, GT, 128], F32, tag="oT")
            for e in range(E):
                nc.tensor.matmul(
                    oT[:],
                    al_sb[:, e, :],
                    xeT[:, GT * e:GT * (e + 1), :],
                    start=(e == 0),
                    stop=(e == E - 1),
                )

            # 4. evict + out DMA (ACT ring)
            o_sb = osp.tile([128, GT, 128], F16, tag="o")
            nc.scalar.copy(o_sb[:], oT[:])
            nc.scalar.dma_start(
                outT[:, G * GROUP_COLS:(G + 1) * GROUP_COLS],
                o_sb[:].rearrange("u t q -> u (t q)"),
            )

    nc.compile()
    return nc


_NC_CACHE = None


def make_in_maps(x, gate_perc, alpha, beta=None):
    x16 = np.asarray(x, dtype=np.float16)
    g16 = np.asarray(gate_perc, dtype=np.float16)
    al16 = np.ascontiguousarray(
        np.asarray(alpha, dtype=np.float16).transpose(1, 0, 2)
    )
    in_maps = []
    for c in range(NCORES):
        sl = slice(c * BLOC, (c + 1) * BLOC)
        # g2[q, T, e, j] = gate[cBLOC + 128T + q, e]
        gc = g16[sl].reshape(NT, 128, E).transpose(1, 0, 2)  # [q, T, e]
        g2 = np.ascontiguousarray(
            np.broadcast_to(gc[:, :, :, None], (128, NT, E, 2)).astype(np.float16)
        )
        in_maps.append(
            {
                "x16": np.ascontiguousarray(x16[sl]),
                "g2": g2,
                "al": al16,
            }
        )
    return in_maps


def assemble(results, gate_perc, beta):
    # per-core outT is [U, BLOC] f16 with column b = batch row within core
    full_T = np.concatenate([results[c]["outT"] for c in range(NCORES)], axis=1)
    out = np.ascontiguousarray(full_T.T).astype(np.float32)
    out += np.asarray(gate_perc, dtype=np.float32) @ np.asarray(beta, dtype=np.float32)
    return out


def kernel(x, gate_perc, alpha, beta):
    global _NC_CACHE
    if _NC_CACHE is None:
        _NC_CACHE = _build()
    nc = _NC_CACHE

    in_maps = make_in_maps(x, gate_perc, alpha)
    res = run_bass_kernel_spmd(nc, in_maps, list(range(NCORES))).results
    return assemble(res, gate_perc, beta)


if __name__ == "__main__":
    rng = np.random.default_rng(0)
    x = rng.standard_normal((B, I)).astype(np.float32)
    g = rng.random((B, E)).astype(np.float32)
    g /= g.sum(-1, keepdims=True)
    al = (rng.standard_normal((E, I, U)) * 0.05).astype(np.float32)
    be = (rng.standard_normal((E, U)) * 0.05).astype(np.float32)
    got = kernel(x, g, al, be)
    ref = np.einsum("bi,eio->beo", x, al, optimize=True)
    ref = np.einsum("beo,be->bo", ref, g) + g @ be
    err = np.abs(got - ref)
    print("max abs err", err.max(), "rel", err.max() / np.abs(ref).max())


# revision 9
# speedup vs baseline: 1.0610x; 1.0610x over previous
"""Trainium2 Bass kernel for nn_DenseExpert (soft-gated mixture of dense experts).

Math:  out[b,u] = sum_e gate[b,e] * (x[b,:] @ alpha[e]) [u] + (gate @ beta)[b,u]

Hybrid strategy (pure data parallel over batch, 8 cores; BLOC=8192/core).
The bias term (gate @ beta, 1.5% of FLOPs) is added on the host.

Per 512-row group (16 groups of 4 tiles x 128 rows), the 8 experts split
across two scaled-transpose pipelines that run on different hardware:

  Experts 0-3 ("xbar path"):
    DVE gate-scales x into 4 expert copies (tensor_tensor in 2x_1p mode
    via a host-duplicated gate pair layout), then ONE xbar-DMA transpose
    per group turns [q, (e,t), i] into [i, (e,t), q] off the compute
    engines. (The xbar is a serial device: keep all transposes on one
    ring - concurrent transposes from two rings corrupt data.)

  Experts 4-7 ("diag path", PE does the scale+transpose):
    The host precomputes dstk[q, T, d, c] = gate[128T+q, 4+d] * [c==q%64].
    Two matmuls per tile (x block-stationary, dstk moving) produce
    yT[i, (d,c)] = gate * x transposed directly in PSUM; ACT gathers
    PSUM->SBUF with f32->f16 cast.

  Mains: 8 accumulating matmuls per group (N=512):
    oT[u, (t,q)] += alpha_e[i,u].T @ scaled-xT_e.
  DVE evicts oT (f32->f16); out DMA per group on the scalar ring.

Host: x/gate cast to f16, dstk/g2 packing, alpha pre-transposed to
[i, e, u]; output [u, b] f16 transposed back, cast to f32, bias added.
"""

import dataclasses
from contextlib import ExitStack

import numpy as np

import concourse.bacc as bacc
import concourse.tile as tile
import concourse.mybir as mybir
from concourse.bass_utils import run_bass_kernel_spmd

F32 = mybir.dt.float32
F16 = mybir.dt.float16

B, E, I, U = 65536, 8, 128, 128
NCORES = 8
BLOC = B // NCORES        # 8192 rows per core
NT = BLOC // 128          # 64 tiles of 128 rows
GT = 4                    # tiles per group
NG = NT // GT             # 16 groups
GROUP_COLS = GT * 128     # 512 columns of out.T per group
KX = 4                    # experts on the xbar path (0..KX-1)
MD = E - KX               # experts on the PE-diag path (KX..E-1)
KB = 128                  # diag block size (full tile; c == q)


def _build():
    nc = bacc.Bacc("TRN2", target_bir_lowering=False, debug=False)

    x16 = nc.dram_tensor("x16", [BLOC, I], F16, kind="ExternalInput").ap()
    # g2[q, T, e, j] = gate[128T+q, e] duplicated over j in {0,1}
    g2 = nc.dram_tensor("g2", [128, NT, KX, 2], F16, kind="ExternalInput").ap()
    # al[i, e, u] = alpha[e, i, u]
    al = nc.dram_tensor("al", [128, E, U], F16, kind="ExternalInput").ap()
    # dstk[q, T, d, c] = gate[128T+q, KX+d] * (c == q)
    dstk = nc.dram_tensor("dstk", [128, NT, MD, KB], F16, kind="ExternalInput").ap()
    # out.T, columns ordered (T, q): col = 128T + q = batch row
    outT = nc.dram_tensor("outT", [U, BLOC], F16, kind="ExternalOutput").ap()

    with tile.TileContext(nc) as tc, ExitStack() as ctx:
        const = ctx.enter_context(tc.tile_pool(name="const", bufs=1))
        xep = ctx.enter_context(tc.tile_pool(name="xep", bufs=3))
        xtp = ctx.enter_context(tc.tile_pool(name="xtp", bufs=3))
        ytp = ctx.enter_context(tc.tile_pool(name="ytp", bufs=3))
        osp = ctx.enter_context(tc.tile_pool(name="osp", bufs=2))
        psy = ctx.enter_context(tc.tile_pool(name="psy", bufs=2, space="PSUM"))
        pso = ctx.enter_context(tc.tile_pool(name="pso", bufs=2, space="PSUM"))

        x_all = const.tile([128, NT, I], F16, tag="x")
        for q4 in range(4):
            nc.sync.dma_start(
                x_all[:, q4 * 16:(q4 + 1) * 16, :],
                x16[q4 * 2048:(q4 + 1) * 2048, :].rearrange(
                    "(T p) i -> p T i", p=128
                ),
            )
        g_sb = const.tile([128, NT, KX, 2], F16, tag="g")
        nc.sync.dma_start(g_sb[:], g2)
        al_sb = const.tile([128, E, U], F16, tag="al")
        nc.sync.dma_start(al_sb[:], al)
        dstk_sb = const.tile([128, NT, MD, KB], F16, tag="dstk")
        for q4 in range(4):
            nc.scalar.dma_start(
                dstk_sb[:, q4 * 16:(q4 + 1) * 16, :, :],
                dstk[:, q4 * 16:(q4 + 1) * 16, :, :],
            )

        g_pitch = NT * KX * 2
        for G in range(NG):
            # --- xbar path: gate-scale experts 0..KX-1, one DVE op per tile
            xe = xep.tile([128, KX, GT, I], F16, tag="xe")
            for t in range(GT):
                T = G * GT + t
                xe_v = dataclasses.replace(
                    xe[:],
                    ap=[[KX * GT * I, 128], [GT * I, KX], [2, 64], [1, 2]],
                    offset=t * I,
                )
                x_v = dataclasses.replace(
                    x_all[:],
                    ap=[[NT * I, 128], [0, KX], [2, 64], [1, 2]],
                    offset=T * I,
                )
                g_v = dataclasses.replace(
                    g_sb[:],
                    ap=[[g_pitch, 128], [2, KX], [0, 64], [1, 2]],
                    offset=T * KX * 2,
                )
                nc.vector.tensor_tensor(xe_v, x_v, g_v, op=mybir.AluOpType.mult)

            # xbar transpose (single ring!): xeT[i, (e,t), q]
            xeT = xtp.tile([128, KX * GT, 128], F16, tag="xeT")
            xe2d = dataclasses.replace(
                xe[:], ap=[[KX * GT * I, 128], [1, KX * GT * I]], offset=0
            )
            nc.sync.dma_start_transpose(xeT[:], xe2d)

            # --- diag path: experts KX..E-1 via PE (KB=128: one matmul per
            # tile, full-partition stationary x and moving dstk; column c==q)
            # yT_sb layout [t, d, c]; psum per tile-pair [t2, (d c)]
            yT_sb = ytp.tile([128, GT, MD, KB], F16, tag="yT")
            for tp in range(2):
                yps = psy.tile([128, 2, MD * KB], F32, tag="yps")
                for t2 in range(2):
                    T = G * GT + 2 * tp + t2
                    nc.tensor.matmul(
                        yps[:, t2, :],
                        x_all[:, T, :],
                        dstk_sb[:, T, :, :],
                        start=True,
                        stop=True,
                    )
                # gather PSUM -> SBUF (ACT): flat contiguous copy
                gdst = dataclasses.replace(
                    yT_sb[:],
                    ap=[[GT * MD * KB, 128], [1, 2 * MD * KB]],
                    offset=tp * 2 * MD * KB,
                )
                gsrc = dataclasses.replace(
                    yps[:],
                    ap=[[2 * MD * KB, 128], [1, 2 * MD * KB]],
                    offset=0,
                )
                nc.scalar.copy(gdst, gsrc)

            # --- mains: oT[u, (t, q)] accumulated over all 8 experts
            oT = pso.tile([128, GT, 128], F32, tag="oT")
            for e in range(KX):
                nc.tensor.matmul(
                    oT[:],
                    al_sb[:, e, :],
                    xeT[:, GT * e:GT * (e + 1), :],
                    start=(e == 0),
                    stop=False,
                )
            for d in range(MD):
                mv = dataclasses.replace(
                    yT_sb[:],
                    ap=[[GT * MD * KB, 128], [MD * KB, GT], [1, KB]],
                    offset=d * KB,
                )
                nc.tensor.matmul(
                    oT[:],
                    al_sb[:, KX + d, :],
                    mv,
                    start=False,
                    stop=(d == MD - 1),
                )

            # --- evict (DVE) + out DMA (scalar ring)
            o_sb = osp.tile([128, GT, 128], F16, tag="o")
            nc.vector.tensor_copy(o_sb[:], oT[:])
            nc.scalar.dma_start(
                outT[:, G * GROUP_COLS:(G + 1) * GROUP_COLS],
                o_sb[:].rearrange("u t q -> u (t q)"),
            )

    nc.compile()
    return nc


_NC_CACHE = None


def make_in_maps(x, gate_perc, alpha, beta=None):
    x16 = np.asarray(x, dtype=np.float16)
    g16 = np.asarray(gate_perc, dtype=np.float16)
    al16 = np.ascontiguousarray(
        np.asarray(alpha, dtype=np.float16).transpose(1, 0, 2)
    )
    in_maps = []
    for c in range(NCORES):
        sl = slice(c * BLOC, (c + 1) * BLOC)
        gc = g16[sl].reshape(NT, 128, E).transpose(1, 0, 2)  # [q, T, e]
        # g2[q, T, e, j] for xbar experts
        g2 = np.ascontiguousarray(
            np.broadcast_to(
                gc[:, :, :KX, None], (128, NT, KX, 2)
            ).astype(np.float16)
        )
        # dstk[q, T, d, c] = gc[q, T, KX+d] * (c == q)
        dstk = np.zeros((128, NT, MD, KB), np.float16)
        dstk[np.arange(128)[:, None, None],
             np.arange(NT)[None, :, None],
             np.arange(MD)[None, None, :],
             np.arange(128)[:, None, None]] = gc[:, :, KX:]
        in_maps.append(
            {
                "x16": np.ascontiguousarray(x16[sl]),
                "g2": g2,
                "al": al16,
                "dstk": dstk,
            }
        )
    return in_maps


def assemble(results, gate_perc, beta):
    # per-core outT is [U, BLOC] f16 with column b = batch row within core
    full_T = np.concatenate([results[c]["outT"] for c in range(NCORES)], axis=1)
    out = np.ascontiguousarray(full_T.T).astype(np.float32)
    out += np.asarray(gate_perc, dtype=np.float32) @ np.asarray(beta, dtype=np.float32)
    return out


def kernel(x, gate_perc, alpha, beta):
    global _NC_CACHE
    if _NC_CACHE is None:
        _NC_CACHE = _build()
    nc = _NC_CACHE

    in_maps = make_in_maps(x, gate_perc, alpha)
    res = run_bass_kernel_spmd(nc, in_maps, list(range(NCORES))).results
    return assemble(res, gate_perc, beta)


if __name__ == "__main__":
    rng = np.random.default_rng(0)
    x = rng.standard_normal((B, I)).astype(np.float32)
    g = rng.random((B, E)).astype(np.float32)
    g /= g.sum(-1, keepdims=True)
    al = (rng.standard_normal((E, I, U)) * 0.05).astype(np.float32)
    be = (rng.standard_normal((E, U)) * 0.05).astype(np.float32)
    got = kernel(x, g, al, be)
    ref = np.einsum("bi,eio->beo", x, al, optimize=True)
    ref = np.einsum("beo,be->bo", ref, g) + g @ be
    err = np.abs(got - ref)
    print("max abs err", err.max(), "rel", err.max() / np.abs(ref).max())
